# revision 76
# baseline (speedup 1.0000x reference)
"""Trainium2 Bass kernel for nn_CLNF_54769422959177.

Computes (dp, dw) where dp = vf(p) (4-layer VectorField MLP forward) and
dw = -vjp(vf, p)(w), data-parallel over 8 NeuronCores.

v6 design (1178909 -> 578819 ns vs the v1 baseline):
- A single manual InstLoadActFuncSet(natural_log_exp_and_others) at program
  start: every ACT func used (Exp/Ln/Copy) lives in that one table, so the
  finalize pass inserts no further table loads (v1 thrashed 443 loads
  = 568us on the ACT engine).
- LN stats via per-block bn_stats/bn_aggr (mean+var in one DVE pass)
  instead of Square + two reduces; rstd = exp(-0.5*ln(var+eps)).
- Backward in xhat-form: dz = (gd - xhat*m)*rstd*sigmoid, with m2 from
  scalar_tensor_tensor+accum and sigmoid = 1 - exp(-A) from the saved
  softplus output A (no recompute of the forward pre-activation).
- fp16 tensor_scalar ops (4x DVE mode) for xhat/srn/xm with per-block
  [128,1] stat scalars; engine split: ACT = exp/ln + PSUM evacuations,
  DVE = bn/stt/ts/tt + transpose copies, Pool = dz + small stat ops.
- f32 entry: p and w are PE-transposed and matmul'ed in f32 directly (PE
  has slack), skipping the f32->fp16 cast pass of v1.
- NS=4 phase-staggered free-running streams; one PSUM bank per stream per
  direction (entry transposes, matmuls and activation transposes share a
  single-buffer ring - all chain-serial within a stream); batched bias
  matmul (one wide K=1 matmul, no ones<->Xst ldweights ping-pong); per-block
  instruction quads split/interleaved across yields so the 4-deep in-order
  engine wait queues don't head-of-line block on one late dependency;
  output-store DMAs issued from the idle gpsimd queue so neither the SP
  nor the ACT sequencer parks on result availability.
"""

import numpy as np

import concourse.bass as bass
from concourse import bacc
import concourse.tile as tile
from concourse import mybir
from concourse.bass_utils import run_bass_kernel_spmd

B, D, H, L = 131072, 128, 128, 4
NCORES = 8
LN_EPS = 1e-5
FP16 = mybir.dt.float16
F32 = mybir.dt.float32
AF = mybir.ActivationFunctionType
OP = mybir.AluOpType
ACT_TABLE_NL_EXP = 6  # natural_log_exp_and_others in cayman act_info.json

TileCtx = tile.TileContext


def _emit(nc, R, GBLK):
    """Emit the per-core program: R rows, blocks of 128 rows, GBLK blocks/group."""
    NG = R // (GBLK * 128)
    assert NG * GBLK * 128 == R

    p_in = nc.dram_tensor("p", [R, D], F32, kind="ExternalInput")
    w_in = nc.dram_tensor("w", [R, D], F32, kind="ExternalInput")
    # moving weights fwd: [K, 5, N] = {W_in.T, Wg0.T, Wg1.T, Wg2.T, M2.T}
    wf_in = nc.dram_tensor("wf", [128, 5, 128], FP16, kind="ExternalInput")
    wf0_in = nc.dram_tensor("wf0", [128, 128], F32, kind="ExternalInput")
    # moving weights bwd: {M2c, Wc2, Wc1, Wc0, W_in}
    wb_in = nc.dram_tensor("wb", [128, 5, 128], FP16, kind="ExternalInput")
    wb0_in = nc.dram_tensor("wb0", [128, 128], F32, kind="ExternalInput")
    cb_in = nc.dram_tensor("cb", [1, 5, GBLK * 128], FP16, kind="ExternalInput")
    cb0_in = nc.dram_tensor("cb0", [1, GBLK * 128], F32, kind="ExternalInput")
    ones_in = nc.dram_tensor("ones1", [1, 128], FP16, kind="ExternalInput")
    ones32_in = nc.dram_tensor("ones1_32", [1, 128], F32, kind="ExternalInput")
    id_in = nc.dram_tensor("ident", [128, 128], FP16, kind="ExternalInput")
    id32_in = nc.dram_tensor("ident32", [128, 128], F32, kind="ExternalInput")
    dp_out = nc.dram_tensor("dp", [R, D], F32, kind="ExternalOutput")
    dw_out = nc.dram_tensor("dw", [R, D], F32, kind="ExternalOutput")

    pv = p_in[:, :].rearrange("(g b p) d -> g p b d", p=128, b=GBLK)
    wv = w_in[:, :].rearrange("(g b p) d -> g p b d", p=128, b=GBLK)
    dpv = dp_out[:, :].rearrange("(g b p) d -> g p b d", p=128, b=GBLK)
    dwv = dw_out[:, :].rearrange("(g b p) d -> g p b d", p=128, b=GBLK)

    from contextlib import ExitStack

    with TileCtx(nc) as tc, ExitStack() as ctx:
        NS = 4
        consts = ctx.enter_context(tc.tile_pool(name="consts", bufs=1))
        io = ctx.enter_context(tc.tile_pool(name="io", bufs=2))
        work = ctx.enter_context(tc.tile_pool(name="work", bufs=1))
        saves = ctx.enter_context(tc.tile_pool(name="saves", bufs=2))
        stats = ctx.enter_context(tc.tile_pool(name="stats", bufs=2))
        # One PSUM bank per stream per direction; entry transposes, layer
        # matmuls and activation transposes all rotate through the same
        # single-buffer ring (their uses are chain-serial within a stream).
        zpf = [
            ctx.enter_context(tc.tile_pool(name=f"zpf{s}", bufs=1, space="PSUM"))
            for s in range(NS)
        ]
        zpb = [
            ctx.enter_context(tc.tile_pool(name=f"zpb{s}", bufs=1, space="PSUM"))
            for s in range(NS)
        ]

        wfs = consts.tile([128, 5, 128], FP16, tag="wfs")
        wf0 = consts.tile([128, 128], F32, tag="wf0")
        wbs = consts.tile([128, 5, 128], FP16, tag="wbs")
        wb0 = consts.tile([128, 128], F32, tag="wb0")
        cbs = consts.tile([1, 5, GBLK * 128], FP16, tag="cbs")
        cb0 = consts.tile([1, GBLK * 128], F32, tag="cb0")
        ones1 = consts.tile([1, 128], FP16, tag="ones1")
        ones1_32 = consts.tile([1, 128], F32, tag="ones1_32")
        ident = consts.tile([128, 128], FP16, tag="ident")
        ident32 = consts.tile([128, 128], F32, tag="ident32")
        epsb = consts.tile([128, 1], F32, tag="epsb")
        nc.vector.memset(epsb, LN_EPS)
        nc.gpsimd.dma_start(out=wfs[:], in_=wf_in[:, :, :])
        nc.gpsimd.dma_start(out=wf0[:], in_=wf0_in[:, :])
        nc.gpsimd.dma_start(out=wbs[:], in_=wb_in[:, :, :])
        nc.gpsimd.dma_start(out=wb0[:], in_=wb0_in[:, :])
        nc.gpsimd.dma_start(out=cbs[:], in_=cb_in[:, :, :])
        nc.gpsimd.dma_start(out=cb0[:], in_=cb0_in[:, :])
        nc.gpsimd.dma_start(out=ones1[:], in_=ones_in[:, :])
        nc.gpsimd.dma_start(out=ones1_32[:], in_=ones32_in[:, :])
        nc.gpsimd.dma_start(out=ident[:], in_=id_in[:, :])
        nc.gpsimd.dma_start(out=ident32[:], in_=id32_in[:, :])

        # One activation table covering Exp, Ln, Copy: loaded once, the
        # finalize fixpoint then inserts no per-activation loads.
        ld = mybir.InstLoadActFuncSet(
            name=nc.get_next_instruction_name(), ins=[], outs=[]
        )
        ld.act_func_set_id = ACT_TABLE_NL_EXP
        nc.scalar.add_instruction(ld)

        def emit_fwd(g, s, out):
            """Generator: forward for group g on stream s; appends saves."""
            pf = io.tile([128, GBLK, 128], F32, tag=f"pin{s}")
            nc.sync.dma_start(out=pf, in_=pv[g])
            # entry: f32 transpose + f32 copy to SBUF (no fp16 cast pass)
            tpe = zpf[s].tile([128, GBLK, 128], F32, tag=f"zpf{s}")
            for b in range(GBLK):
                nc.tensor.transpose(tpe[:, b, :], pf[:, b, :], ident32[:, :])
            Xst32 = work.tile([128, GBLK, 128], F32, tag=f"xst32{s}")
            nc.scalar.copy(Xst32, tpe)
            yield

            Xst16 = None
            for i in range(L):
                zp = zpf[s].tile([128, GBLK, 128], F32, tag=f"zpf{s}")
                if i == 0:
                    nc.tensor.matmul(
                        zp[:, :, :], ones1_32[:, :], cb0[:, :],
                        start=True, stop=False, skip_group_check=True,
                    )
                    for b in range(GBLK):
                        nc.tensor.matmul(
                            zp[:, b, :], Xst32[:, b, :], wf0[:, :],
                            start=False, stop=True, skip_group_check=True,
                        )
                else:
                    nc.tensor.matmul(
                        zp[:, :, :], ones1[:, :], cbs[:, i, :],
                        start=True, stop=False, skip_group_check=True,
                    )
                    for b in range(GBLK):
                        nc.tensor.matmul(
                            zp[:, b, :], Xst16[:, b, :], wfs[:, i, :],
                            start=False, stop=True, skip_group_check=True,
                        )

                E = work.tile([128, GBLK, 128], F32, tag=f"E{s}")
                nc.scalar.activation(E, zp, AF.Exp)
                A16 = saves.tile([128, GBLK, 128], FP16, tag=f"A{i}{s}")
                nc.scalar.activation(A16, E, AF.Ln, bias=1.0)
                yield

                st6 = stats.tile([128, GBLK, 6], F32, tag=f"st6{s}")
                aggr = saves.tile([128, GBLK, 2], F32, tag=f"ag{i}{s}")
                for b in range(2):
                    nc.vector.bn_stats(out=st6[:, b, :], in_=A16[:, b, :])
                yield
                for b in range(2, GBLK):
                    nc.vector.bn_stats(out=st6[:, b, :], in_=A16[:, b, :])
                for b in range(2):
                    nc.vector.bn_aggr(out=aggr[:, b, :], in_=st6[:, b, :])
                yield
                for b in range(2, GBLK):
                    nc.vector.bn_aggr(out=aggr[:, b, :], in_=st6[:, b, :])
                lnv = stats.tile([128, GBLK, 1], F32, tag=f"lnv{s}")
                nc.scalar.activation(
                    lnv, aggr[:, :, 1:2], AF.Ln, bias=epsb[:, :]
                )
                rstd = saves.tile([128, GBLK, 1], F32, tag=f"rs{i}{s}")
                nc.scalar.activation(rstd, lnv, AF.Exp, scale=-0.5)
                yield

                xh16 = saves.tile([128, GBLK, 128], FP16, tag=f"xh{i}{s}")
                for b in range(2):
                    nc.vector.tensor_scalar(
                        out=xh16[:, b, :], in0=A16[:, b, :],
                        scalar1=aggr[:, b, 0:1], scalar2=rstd[:, b, :],
                        op0=OP.subtract, op1=OP.mult,
                    )
                yield
                tp = zpf[s].tile([128, GBLK, 128], FP16, tag=f"zpf{s}")
                for b in range(2, GBLK):
                    nc.vector.tensor_scalar(
                        out=xh16[:, b, :], in0=A16[:, b, :],
                        scalar1=aggr[:, b, 0:1], scalar2=rstd[:, b, :],
                        op0=OP.subtract, op1=OP.mult,
                    )
                for b in range(2):
                    nc.tensor.transpose(tp[:, b, :], xh16[:, b, :], ident[:, :])
                yield
                for b in range(2, GBLK):
                    nc.tensor.transpose(tp[:, b, :], xh16[:, b, :], ident[:, :])
                Xst16 = work.tile([128, GBLK, 128], FP16, tag=f"xst{s}")
                nc.vector.tensor_scalar(
                    out=Xst16, in0=tp, scalar1=1.0, scalar2=None, op0=OP.mult
                )
                out.append((A16, aggr, rstd, xh16))
                yield

            zp = zpf[s].tile([128, GBLK, 128], F32, tag=f"zpf{s}")
            nc.tensor.matmul(
                zp[:, :, :], ones1[:, :], cbs[:, 4, :],
                start=True, stop=False, skip_group_check=True,
            )
            for b in range(GBLK):
                nc.tensor.matmul(
                    zp[:, b, :], Xst16[:, b, :], wfs[:, 4, :],
                    start=False, stop=True, skip_group_check=True,
                )
            yo = io.tile([128, GBLK, 128], F32, tag=f"yout{s}", bufs=1)
            nc.scalar.copy(yo, zp)
            # issue the store from gpsimd: its sequencer is idle, and the
            # wait (yo ready) parks neither SP nor ACT
            nc.gpsimd.dma_start(out=dpv[g], in_=yo)
            yield

        def emit_bwd(g, s, sv):
            wf = io.tile([128, GBLK, 128], F32, tag=f"win{s}")
            nc.sync.dma_start(out=wf, in_=wv[g])
            tpe = zpb[s].tile([128, GBLK, 128], F32, tag=f"zpb{s}")
            for b in range(GBLK):
                nc.tensor.transpose(tpe[:, b, :], wf[:, b, :], ident32[:, :])
            Gst32 = work.tile([128, GBLK, 128], F32, tag=f"gst32{s}")
            nc.scalar.copy(Gst32, tpe)
            yield

            Gst16 = None
            for i in range(L - 1, -1, -1):
                A16, aggr, rstd, xh16 = sv[i]
                # u = exp(-A) = 1 - sigmoid of the pre-softplus input;
                # independent of the matmul chain, issue early.
                u16 = work.tile([128, GBLK, 128], FP16, tag=f"u{s}", bufs=2)
                nc.scalar.activation(u16, A16, AF.Exp, scale=-1.0)

                gp = zpb[s].tile([128, GBLK, 128], F32, tag=f"zpb{s}")
                if i == L - 1:
                    for b in range(GBLK):
                        nc.tensor.matmul(
                            gp[:, b, :], Gst32[:, b, :], wb0[:, :],
                            start=True, stop=True,
                        )
                else:
                    for b in range(GBLK):
                        nc.tensor.matmul(
                            gp[:, b, :], Gst16[:, b, :], wbs[:, 3 - i, :],
                            start=True, stop=True,
                        )
                gd16 = work.tile([128, GBLK, 128], FP16, tag=f"gd{s}", bufs=2)
                nc.scalar.copy(gd16, gp)
                yield

                # srn = rstd*u - rstd (indep of gd) interleaved with the
                # m2 = sum(xhat*gd) reduction (indep of u) so the DVE wait
                # queue never fills with one not-ready dependency group.
                srn = work.tile([128, GBLK, 128], FP16, tag=f"srn{s}", bufs=2)
                pr = work.tile([128, GBLK, 128], FP16, tag=f"pr{s}")
                m2 = stats.tile([128, GBLK, 1], F32, tag=f"m2{s}")
                for b in range(2):
                    nc.gpsimd.tensor_scalar(
                        out=srn[:, b, :], in0=u16[:, b, :],
                        scalar1=rstd[:, b, :], scalar2=rstd[:, b, :],
                        op0=OP.mult, op1=OP.subtract,
                    )
                    nc.vector.scalar_tensor_tensor(
                        out=pr[:, b, :], in0=xh16[:, b, :], scalar=1.0 / H,
                        in1=gd16[:, b, :], op0=OP.mult, op1=OP.mult,
                        accum_out=m2[:, b, :],
                    )
                yield
                for b in range(2, GBLK):
                    nc.gpsimd.tensor_scalar(
                        out=srn[:, b, :], in0=u16[:, b, :],
                        scalar1=rstd[:, b, :], scalar2=rstd[:, b, :],
                        op0=OP.mult, op1=OP.subtract,
                    )
                    nc.vector.scalar_tensor_tensor(
                        out=pr[:, b, :], in0=xh16[:, b, :], scalar=1.0 / H,
                        in1=gd16[:, b, :], op0=OP.mult, op1=OP.mult,
                        accum_out=m2[:, b, :],
                    )
                yield

                # xm = xhat*q (4x tensor_scalar), dxn = xm - gd
                xm = work.tile([128, GBLK, 128], FP16, tag=f"xm{s}")
                for b in range(2):
                    nc.vector.tensor_scalar(
                        out=xm[:, b, :], in0=xh16[:, b, :],
                        scalar1=m2[:, b, :], scalar2=None, op0=OP.mult,
                    )
                yield
                for b in range(2, GBLK):
                    nc.vector.tensor_scalar(
                        out=xm[:, b, :], in0=xh16[:, b, :],
                        scalar1=m2[:, b, :], scalar2=None, op0=OP.mult,
                    )
                dxn = work.tile([128, GBLK, 128], FP16, tag=f"dxn{s}", bufs=2)
                nc.vector.tensor_tensor(
                    out=dxn, in0=xm, in1=gd16, op=OP.subtract
                )
                yield
                # dz = dxn * srn = (gd - xhat*m) * rstd * sigmoid
                dz = work.tile([128, GBLK, 128], FP16, tag=f"dz{s}", bufs=2)
                nc.vector.tensor_tensor(out=dz, in0=dxn, in1=srn, op=OP.mult)
                yield
                tp = zpb[s].tile([128, GBLK, 128], FP16, tag=f"zpb{s}")
                for b in range(2):
                    nc.tensor.transpose(tp[:, b, :], dz[:, b, :], ident[:, :])
                yield
                for b in range(2, GBLK):
                    nc.tensor.transpose(tp[:, b, :], dz[:, b, :], ident[:, :])
                Gst16 = work.tile([128, GBLK, 128], FP16, tag=f"gst{s}")
                nc.vector.tensor_scalar(
                    out=Gst16, in0=tp, scalar1=1.0, scalar2=None, op0=OP.mult
                )
                yield

            gp = zpb[s].tile([128, GBLK, 128], F32, tag=f"zpb{s}")
            for b in range(GBLK):
                nc.tensor.matmul(
                    gp[:, b, :], Gst16[:, b, :], wbs[:, 4, :],
                    start=True, stop=True,
                )
            dwo = io.tile([128, GBLK, 128], F32, tag=f"dwout{s}", bufs=1)
            nc.scalar.copy(dwo, gp)
            nc.gpsimd.dma_start(out=dwv[g], in_=dwo)
            yield

        # Free-running phase-staggered pipeline: each stream s processes
        # groups s, s+NS, ... as one continuous fwd->bwd chain; streams are
        # primed with an emission-offset so their phases stay staggered and
        # every engine always sees ready work from some stream.
        assert NG % NS == 0

        def stream_gen(s):
            for g in range(s, NG, NS):
                sv = []
                yield from emit_fwd(g, s, sv)
                yield from emit_bwd(g, s, sv)

        gens = [iter(stream_gen(s)) for s in range(NS)]
        live = []
        PRIME = 5  # chunks of head-start between adjacent streams
        for s in range(NS):
            live.append(gens[s])
            for it in list(live):
                for _ in range(PRIME if it is gens[s] else 1):
                    try:
                        next(it)
                    except StopIteration:
                        if it in live:
                            live.remove(it)
                        break
        while live:
            for it in list(live):
                try:
                    next(it)
                except StopIteration:
                    live.remove(it)


def _host_precompute(t, W_in, b_in, fw, fb, gamma, beta, Wl, bl, W_out, b_out):
    t = np.asarray(t, dtype=np.float32).reshape(-1)[0]
    s = np.sin(t * np.asarray(fw, np.float32) + np.asarray(fb, np.float32))  # [L, H]
    Wl = np.asarray(Wl, np.float32)
    gamma = np.asarray(gamma, np.float32)
    beta = np.asarray(beta, np.float32)
    bl = np.asarray(bl, np.float32)
    W_in = np.asarray(W_in, np.float32)
    W_out = np.asarray(W_out, np.float32)
    b_in = np.asarray(b_in, np.float32)
    b_out = np.asarray(b_out, np.float32)

    Wg = [Wl[i] * gamma[i][None, :] for i in range(L)]          # [H, H]
    bg = [bl[i] + Wl[i] @ beta[i] for i in range(L)]            # [H]

    # fuse h4->y: y = xhat3 @ (W_out@Wg3).T + (b_out + W_out@bg3)
    M2 = (W_out.astype(np.float64) @ Wg[L - 1].astype(np.float64)).astype(np.float32)
    c = np.zeros((5, 128), np.float32)
    c[0] = b_in + s[0]
    for i in range(1, L):
        c[i] = bg[i - 1] + s[i]
    c[4] = b_out + W_out @ bg[L - 1]
    WF = np.stack(
        [W_in.T] + [Wg[i].T for i in range(L - 1)] + [M2.T], axis=0
    )  # [5, K, N]
    Wc = [Wg[i] - Wg[i].mean(axis=1, keepdims=True) for i in range(L - 1)]
    M2n = -M2
    M2c = M2n - M2n.mean(axis=1, keepdims=True)
    WB = np.stack([M2c, Wc[2], Wc[1], Wc[0], W_in], axis=0)

    WF16 = np.ascontiguousarray(np.transpose(WF, (1, 0, 2))).astype(np.float16)
    WF0 = np.ascontiguousarray(W_in.T)  # [K, N] f32
    WB16 = np.ascontiguousarray(np.transpose(WB, (1, 0, 2))).astype(np.float16)
    WB0 = np.ascontiguousarray(M2c)  # f32
    GBLK = 4
    CB = np.tile(c, (1, GBLK)).astype(np.float16)[None, :, :]  # [1, 5, GBLK*128]
    CB0 = np.tile(c[0:1, :], (1, GBLK)).astype(np.float32)     # [1, GBLK*128]
    ONES = np.ones((1, 128), np.float16)
    ONES32 = np.ones((1, 128), np.float32)
    EYE = np.eye(128, dtype=np.float16)
    EYE32 = np.eye(128, dtype=np.float32)
    return WF16, WF0, WB16, WB0, CB, CB0, ONES, ONES32, EYE, EYE32


_NC_CACHE = {}


def _get_nc(R, GBLK):
    key = (R, GBLK)
    if key not in _NC_CACHE:
        nc = bacc.Bacc("TRN2")
        _emit(nc, R, GBLK)
        nc.finalize()
        _NC_CACHE[key] = nc
    return _NC_CACHE[key]


def _run(p, w, consts, R, GBLK, n_cores):
    WF16, WF0, WB16, WB0, CB, CB0, ONES, ONES32, EYE, EYE32 = consts
    nc = _get_nc(R, GBLK)
    in_maps = []
    for k in range(n_cores):
        in_maps.append(
            {
                "p": np.ascontiguousarray(p[k * R : (k + 1) * R]),
                "w": np.ascontiguousarray(w[k * R : (k + 1) * R]),
                "wf": WF16,
                "wf0": WF0,
                "wb": WB16,
                "wb0": WB0,
                "cb": CB,
                "cb0": CB0,
                "ones1": ONES,
                "ones1_32": ONES32,
                "ident": EYE,
                "ident32": EYE32,
            }
        )
    res = run_bass_kernel_spmd(nc, in_maps, core_ids=list(range(n_cores)))
    dp = np.concatenate([r["dp"] for r in res.results], axis=0)
    dw = np.concatenate([r["dw"] for r in res.results], axis=0)
    return dp, dw


def kernel(t, p, w, W_in, b_in, fw, fb, gamma, beta, Wl, bl, W_out, b_out):
    consts = _host_precompute(
        t, W_in, b_in, fw, fb, gamma, beta, Wl, bl, W_out, b_out
    )
    p = np.asarray(p, np.float32)
    w = np.asarray(w, np.float32)
    R = p.shape[0] // NCORES
    dp, dw = _run(p, w, consts, R, GBLK=4, n_cores=NCORES)
    return dp, dw
